# revision 19
# baseline (speedup 1.0000x reference)
"""Block-diagonal linear (grouped GEMM) on 8 TRN2 NeuronCores.

out[b, g*512+n] = sum_k x[b, g*512+k] * blocks[g, k, n]

Sharding: group-parallel — core g computes block g's GEMM. The host hands
each core xT = x[:, g*512:(g+1)*512].T ([512, 8192], feature-major) in
bf16 and receives outT ([512, 8192]) in bf16; transposes and dtype
conversion happen on the host so the device needs no PE transposes and
every DMA stream reads/writes long contiguous runs per partition.

bf16 halves HBM traffic vs fp32 (the fp32 version was DMA-bound at
~34.6MB/core ≈ 96µs; bf16 is ~17.3MB ≈ 48µs) while the PE runs bf16 at
the same 1 cycle/row as f32r, so the kernel becomes compute-bound at
~55µs. Accumulation stays fp32 in PSUM; end-to-end max rel err vs the
fp32 reference is ~4e-3 (gate 2e-2).

Per-core kernel: out.T = W.T @ x.T as 64 PSUM accumulation groups:
psum[n-tile 128, m 512] += W[k-tile, n-tile].T @ xT[k-tile, m-chunk].
"""
import numpy as np
import ml_dtypes

import concourse.bacc as bacc
import concourse.tile as tile
from concourse import mybir
from concourse.bass_utils import run_bass_kernel_spmd

TOKENS = 8192
G = 8
M = 512  # per-block in-features
N = 512  # per-block out-features
P = 128
KT = M // P  # 4 contraction tiles
NT = N // P  # 4 output feature tiles
SUB = 512    # tokens per PSUM group (one fp32 PSUM bank)
F32 = mybir.dt.float32
BF16 = mybir.dt.bfloat16
NPBF16 = ml_dtypes.bfloat16

# token-chunk schedule: tiny head so the first PSUM group's data lands as
# early as possible, 2048 steady, tapered tail so the last casts/DMAs drain
# right behind the last matmul
CHUNKS = [128, 384, 512, 1024, 2048, 2048, 1024, 512, 384, 128]
assert sum(CHUNKS) == TOKENS
CMAX = max(CHUNKS)
N_WARM = 45  # dummy matmuls that keep the PE busy while the first DMAs land

_CACHE: dict = {}


def _body(tc, nc, xT, w, outT):
    with (
        tc.tile_pool(name="wp", bufs=1) as wp,
        tc.tile_pool(name="xin", bufs=21) as xin,
        tc.tile_pool(name="outp", bufs=2) as outp,
        tc.tile_pool(name="pso", bufs=4, space="PSUM") as pso,
    ):
        # weights [512, 512] bf16 -> [128, kt, 512]
        w_r = wp.tile([P, KT, N], BF16, tag="wr")
        w_v = w.rearrange("(j p) n -> j p n", p=P)

        # DMA issue plan.  x k-tile j of a chunk goes to ring j%2 (sync=SP /
        # scalar=ACT HWDGE).  Issues for chunk ci are emitted PF chunks ahead
        # of ci's compute so they never queue behind the PSUM casts those
        # engines also run (in-order engine queues: a cast waiting on matmuls
        # would otherwise block later DMA issues and starve the input).
        PF = 3

        def issue_x(ci):
            c, m0 = CHUNKS[ci], sum(CHUNKS[:ci])
            xs = []
            for j in range(KT):
                x_t = xin.tile([P, CMAX], BF16, tag="x", name=f"x{ci}_{j}")
                eng = nc.sync if j % 2 == 0 else nc.scalar
                eng.dma_start(x_t[:, :c], xT[j * P:(j + 1) * P, m0:m0 + c])
                xs.append(x_t)
            return xs

        # W's first two k-tiles go out first (the first PSUM group's j0/j1
        # matmuls need them), then chunk 0's x, then the rest of W.
        nc.sync.dma_start(w_r[:, 0, :], w_v[0])
        nc.scalar.dma_start(w_r[:, 1, :], w_v[1])
        xq = [issue_x(0)]
        nc.sync.dma_start(w_r[:, 2, :], w_v[2])
        nc.scalar.dma_start(w_r[:, 3, :], w_v[3])
        for ci in range(1, PF):
            xq.append(issue_x(ci))

        # HAM warm-up: the PE only reaches full clock after ~3.4us of
        # sustained busy.  Burn that window on dependency-free dummy matmuls
        # over uninitialized SBUF into a scratch PSUM bank (never read) while
        # the first real DMAs are still in flight, so the real matmul stream
        # starts at full rate.
        warm_x = xin.tile([P, CMAX], BF16, tag="x")
        warm_ps = pso.tile([P, 2 * SUB], F32, tag="pso")
        nc.vector.memset(warm_x[:, :2 * P], 0)
        for _ in range(N_WARM):
            nc.tensor.matmul(
                warm_ps[:, :P], warm_x[:, :P], warm_x[:, P:2 * P],
                start=True, stop=True,
            )

        # outT rows viewed as [nt, p, tokens] so one 3D DMA flushes a chunk
        outT_v = outT.rearrange("(nt p) t -> p nt t", p=P)

        m0 = 0
        gi = 0  # PSUM-pair counter, for cast engine striping
        for ci, c in enumerate(CHUNKS):
            # prefetch: issue chunk ci+PF's x DMAs before ci's compute
            if ci + PF < len(CHUNKS):
                xq.append(issue_x(ci + PF))
            xs = xq[ci]

            ot = outp.tile([P, NT, CMAX], BF16, tag="o", name=f"ot{ci}")
            # two adjacent 512-token PSUM groups share a 2-bank tile, so one
            # cast covers both (the casts, split over DVE and ACT, are nearly
            # as expensive as the matmul stream)
            for p0 in range(0, c, 2 * SUB):
                pw = min(2 * SUB, c - p0)
                pss = [
                    pso.tile([P, 2 * SUB], F32, tag="pso", name=f"ps{ci}_{p0}_{nt}")
                    for nt in range(NT)
                ]
                for s0 in range(p0, p0 + pw, SUB):
                    sw = min(SUB, p0 + pw - s0)
                    o = s0 - p0
                    for nt in range(NT):
                        for j in range(KT):
                            nc.tensor.matmul(
                                pss[nt][:, o:o + sw],
                                w_r[:, j, nt * P:(nt + 1) * P],
                                xs[j][:, s0:s0 + sw],
                                start=(j == 0),
                                stop=(j == KT - 1),
                            )
                for nt in range(NT):
                    if gi % 2 == 0:
                        nc.vector.tensor_copy(ot[:, nt, p0:p0 + pw], pss[nt][:, :pw])
                    else:
                        nc.scalar.copy(ot[:, nt, p0:p0 + pw], pss[nt][:, :pw])
                    gi += 1
            # flush the chunk: one fused 3D DMA on the SWDGE ring; the last
            # chunks ride the HWDGE rings (input traffic is done by then)
            if ci >= len(CHUNKS) - 2:
                eng = nc.sync if ci % 2 == 0 else nc.scalar
            else:
                eng = nc.gpsimd
            eng.dma_start(outT_v[:, :, m0:m0 + c], ot[:, :, :c])
            m0 += c


def _build():
    nc = bacc.Bacc("TRN2", target_bir_lowering=False, debug=False, num_devices=G)
    xT = nc.dram_tensor("xT", [M, TOKENS], BF16, kind="ExternalInput").ap()
    w = nc.dram_tensor("w", [M, N], BF16, kind="ExternalInput").ap()
    outT = nc.dram_tensor("outT", [N, TOKENS], BF16, kind="ExternalOutput").ap()
    with tile.TileContext(nc) as tc:
        _body(tc, nc, xT, w, outT)
    nc.compile()
    return nc


def _run(in_maps, **kwargs):
    if "nc" not in _CACHE:
        _CACHE["nc"] = _build()
    return run_bass_kernel_spmd(_CACHE["nc"], in_maps, list(range(G)), **kwargs)


def _in_maps(x, blocks):
    return [
        {
            "xT": np.ascontiguousarray(x[:, g * M:(g + 1) * M].T).astype(NPBF16),
            "w": np.ascontiguousarray(blocks[g]).astype(NPBF16),
        }
        for g in range(G)
    ]


def kernel(x, blocks):
    x = np.asarray(x)
    blocks = np.asarray(blocks)
    res = _run(_in_maps(x, blocks))
    return np.concatenate(
        [res.results[g]["outT"].T.astype(np.float32) for g in range(G)], axis=1
    )


# revision 24
# speedup vs baseline: 1.0521x; 1.0521x over previous
"""Block-diagonal linear (grouped GEMM) on 8 TRN2 NeuronCores.

out[b, g*512+n] = sum_k x[b, g*512+k] * blocks[g, k, n]

Sharding: group-parallel — core g computes block g's GEMM. The host hands
each core xT = x[:, g*512:(g+1)*512].T ([512, 8192], feature-major) in
bf16 and receives outT ([512, 8192]) in bf16; transposes and dtype
conversion happen on the host so the device needs no PE transposes and
every DMA stream reads/writes long contiguous runs per partition.

bf16 halves HBM traffic vs fp32 (the fp32 version was DMA-bound at
~34.6MB/core ≈ 96µs; bf16 is ~17.3MB ≈ 48µs) while the PE runs bf16 at
the same 1 cycle/row as f32r, so the kernel becomes compute-bound at
~55µs. Accumulation stays fp32 in PSUM; end-to-end max rel err vs the
fp32 reference is ~4e-3 (gate 2e-2).

Per-core kernel: out.T = W.T @ x.T as 64 PSUM accumulation groups:
psum[n-tile 128, m 512] += W[k-tile, n-tile].T @ xT[k-tile, m-chunk].
"""
import numpy as np
import ml_dtypes

import concourse.bacc as bacc
import concourse.tile as tile
from concourse import mybir
from concourse.bass_utils import run_bass_kernel_spmd

TOKENS = 8192
G = 8
M = 512  # per-block in-features
N = 512  # per-block out-features
P = 128
KT = M // P  # 4 contraction tiles
NT = N // P  # 4 output feature tiles
SUB = 512    # tokens per PSUM group (one fp32 PSUM bank)
F32 = mybir.dt.float32
BF16 = mybir.dt.bfloat16
NPBF16 = ml_dtypes.bfloat16

# token-chunk schedule: tiny head so the first PSUM group's data lands as
# early as possible, 2048 steady, tapered tail so the last casts/DMAs drain
# right behind the last matmul
CHUNKS = [128, 384, 512, 1024, 2048, 2048, 1024, 512, 384, 128]
assert sum(CHUNKS) == TOKENS
CMAX = max(CHUNKS)
N_WARM = 34  # dummy matmuls that keep the PE busy while the first DMAs land

_CACHE: dict = {}


def _body(tc, nc, xT, w, outT):
    with (
        tc.tile_pool(name="wp", bufs=1) as wp,
        tc.tile_pool(name="xin", bufs=21) as xin,
        tc.tile_pool(name="outp", bufs=3) as outp,
        tc.tile_pool(name="pso", bufs=8, space="PSUM") as pso,
    ):
        # weights [512, 512] bf16 -> [128, kt, 512]
        w_r = wp.tile([P, KT, N], BF16, tag="wr")
        w_v = w.rearrange("(j p) n -> j p n", p=P)

        # DMA issue plan.  x k-tile j of a chunk goes to ring j%2 (sync=SP /
        # scalar=ACT HWDGE).  Issues for chunk ci are emitted PF chunks ahead
        # of ci's compute so they never queue behind the PSUM casts those
        # engines also run (in-order engine queues: a cast waiting on matmuls
        # would otherwise block later DMA issues and starve the input).
        PF = 3

        def issue_x(ci):
            c, m0 = CHUNKS[ci], sum(CHUNKS[:ci])
            xs = []
            for j in range(KT):
                x_t = xin.tile([P, CMAX], BF16, tag="x", name=f"x{ci}_{j}")
                if ci < 4 and j >= 2:
                    # ramp chunks: j2/j3 ride the SWDGE ring, which idles
                    # until the first output flush — halves the load on the
                    # HWDGE rings exactly when the PE is catching up
                    eng = nc.gpsimd
                else:
                    eng = nc.sync if j % 2 == 0 else nc.scalar
                eng.dma_start(x_t[:, :c], xT[j * P:(j + 1) * P, m0:m0 + c])
                xs.append(x_t)
            return xs

        # W's first two k-tiles go out first on the HWDGE rings (the first
        # PSUM group's j0/j1 matmuls need them); w2/w3 ride the idle SWDGE
        # ring so they never queue behind the x stream.
        nc.sync.dma_start(w_r[:, 0, :], w_v[0])
        nc.scalar.dma_start(w_r[:, 1, :], w_v[1])
        nc.gpsimd.dma_start(w_r[:, 2, :], w_v[2])
        nc.gpsimd.dma_start(w_r[:, 3, :], w_v[3])
        xq = [issue_x(ci) for ci in range(PF)]

        # HAM warm-up: the PE only reaches full clock after ~3.4us of
        # sustained busy.  Burn that window on dependency-free dummy matmuls
        # over uninitialized SBUF into a scratch PSUM bank (never read) while
        # the first real DMAs are still in flight, so the real matmul stream
        # starts at full rate.
        warm_x = xin.tile([P, CMAX], BF16, tag="x")
        warm_ps = pso.tile([P, SUB], F32, tag="pso")
        nc.vector.memset(warm_x[:, :2 * P], 0)
        for _ in range(N_WARM):
            nc.tensor.matmul(
                warm_ps[:, :P], warm_x[:, :P], warm_x[:, P:2 * P],
                start=True, stop=True,
            )

        # outT rows viewed as [nt, p, tokens] so one 3D DMA flushes a chunk
        outT_v = outT.rearrange("(nt p) t -> p nt t", p=P)

        m0 = 0
        gi = 0  # PSUM-pair counter, for cast engine striping
        for ci, c in enumerate(CHUNKS):
            # prefetch: issue chunk ci+PF's x DMAs before ci's compute
            if ci + PF < len(CHUNKS):
                xq.append(issue_x(ci + PF))
            xs = xq[ci]

            ot = outp.tile([P, NT, CMAX], BF16, tag="o", name=f"ot{ci}")
            for s0 in range(0, c, SUB):
                sw = min(SUB, c - s0)
                for nt in range(NT):
                    ps_o = pso.tile([P, SUB], F32, tag="pso", name=f"ps{ci}_{s0}_{nt}")
                    for j in range(KT):
                        nc.tensor.matmul(
                            ps_o[:, :sw],
                            w_r[:, j, nt * P:(nt + 1) * P],
                            xs[j][:, s0:s0 + sw],
                            start=(j == 0),
                            stop=(j == KT - 1),
                        )
                    # stripe the PSUM->SBUF casts over DVE and ACT: either
                    # alone is barely slower than the matmul stream
                    if gi % 2 == 0:
                        nc.vector.tensor_copy(ot[:, nt, s0:s0 + sw], ps_o[:, :sw])
                    else:
                        nc.scalar.copy(ot[:, nt, s0:s0 + sw], ps_o[:, :sw])
                    gi += 1
            # flush the chunk: one fused 3D DMA on the SWDGE ring; the last
            # chunks ride the HWDGE rings (input traffic is done by then)
            if ci >= len(CHUNKS) - 2:
                eng = nc.sync if ci % 2 == 0 else nc.scalar
            else:
                eng = nc.gpsimd
            eng.dma_start(outT_v[:, :, m0:m0 + c], ot[:, :, :c])
            m0 += c


def _build():
    nc = bacc.Bacc("TRN2", target_bir_lowering=False, debug=False, num_devices=G)
    xT = nc.dram_tensor("xT", [M, TOKENS], BF16, kind="ExternalInput").ap()
    w = nc.dram_tensor("w", [M, N], BF16, kind="ExternalInput").ap()
    outT = nc.dram_tensor("outT", [N, TOKENS], BF16, kind="ExternalOutput").ap()
    with tile.TileContext(nc) as tc:
        _body(tc, nc, xT, w, outT)
    nc.compile()
    return nc


def _run(in_maps, **kwargs):
    if "nc" not in _CACHE:
        _CACHE["nc"] = _build()
    return run_bass_kernel_spmd(_CACHE["nc"], in_maps, list(range(G)), **kwargs)


def _in_maps(x, blocks):
    return [
        {
            "xT": np.ascontiguousarray(x[:, g * M:(g + 1) * M].T).astype(NPBF16),
            "w": np.ascontiguousarray(blocks[g]).astype(NPBF16),
        }
        for g in range(G)
    ]


def kernel(x, blocks):
    x = np.asarray(x)
    blocks = np.asarray(blocks)
    res = _run(_in_maps(x, blocks))
    return np.concatenate(
        [res.results[g]["outT"].T.astype(np.float32) for g in range(G)], axis=1
    )
